# revision 11
# baseline (speedup 1.0000x reference)
"""Trainium2 Bass kernel for nn_ButterflyRotationLayer (D=4096, M=12).

Math: R = B(d,d) @ B(d,d/2) @ ... @ B(d,2), each B(d,k) a Givens-pair
butterfly factor. Because the support of any column of the partial
product stays inside one half-block at every level, each entry of R is a
SINGLE signed product of 12 cos/sin values (no additions):

    R[r, j] = prod_i F_i(r, j),   i = 0..11, k = 4096 >> i, h = k >> 1
    F_i = sin(theta_i[tidx] + (pi/2) * (1 - rbit + jbit))
    tidx = (j // k) * h + (r & (h - 1))
    rbit = (r >> (11 - i)) & 1,  jbit = (j >> (11 - i)) & 1

Sharding: column-slabs of 512 across 8 cores.  Split at level 3:
    out[r, jj] = A[r] * B[r & 511, jj]        (per core)
where A = prod of levels 0..2 (a 4096-vector; the j-dependence of those
levels is constant inside a 512-column slab) and B = prod of levels
3..11 (a 512x512 local block).  The host gathers thetas into the
F-layout and folds the pi/2 phase in, wrapped to [-pi, pi] (pure index
marshaling + O(d log d) scalar adds); the device applies Sin and runs
the whole O(d^2) product expansion via zero-stride broadcast multiplies.

Schedule (the DMA write of the 8 MiB output at the ~358 GB/s per-core
HBM ceiling is the roofline; everything is ordered to start it early
and keep it saturated):
  - one input DMA; Sin of the wide B11/B10 chunk first (it gates the
    long G1011 -> H chain), the small-chunk Sin overlaps G1011;
  - the whole product chain runs in fp16 (2x DVE throughput; only the
    final output tiles are written f32);
  - output tiles are grouped by t mod 4 (all tiles of class c share
    Btt[c]), so the first group only needs Btt[0] and its DMA issues
    right after the H chain; 7 output DMAs sized small-first, split
    between the Vector and Scalar engines, each group single-producer
    so every HWDGE instruction keeps <= 1 sync wait (this walrus build
    rejects multi-wait instructions);
  - 1 input + 7 output DMAs = exactly the 8 DMA semaphore lanes.
"""

import math
import sys

import numpy as np

sys.path.insert(0, "/opt/trn_rl_repo")

D = 4096
M = 12
NCORES = 8
CPD = D // NCORES  # 512 columns per device
HALF_PI = math.pi / 2.0

# ---------------------------------------------------------------------------
# Factor tile F free-dim coordinates per slice (per core, 128 partitions p):
#   A0: f = t (r = 128t + p);  A1: f = t mod 16;  A2: f = t mod 8
#   B3: f = tt*2 + (jj>>8)  (tt = (r>>7) & 3);  B4: f = (tt&1)*4 + (jj>>7)
#   B5..B11: f = jj >> (11 - level)
# ---------------------------------------------------------------------------

PACK_W = 1088   # width of the factor tile F (f32)

# Layout of the factor tile F, f32 columns.
OFF = {
    "B11": 0, "B10": 512,
    "B3": 768, "B4": 776, "B5": 784, "B6": 792, "B7": 808,
    "B8": 840, "B9": 904,
    "A0": 1032, "A1": 1064, "A2": 1080,
}
# Sin column ranges: the wide B11/B10 chunk first (it gates the
# longest dependent chain G1011 -> H), the small B3..B9+A chunk second.
CHUNKS = ((0, 768), (768, 1088))


def _build_index_tables():
    p = np.arange(128)[:, None]
    lvls, tixs, phps = [], [], []
    for c in range(NCORES):
        lvl = np.zeros((128, PACK_W), np.int64)
        tix = np.zeros((128, PACK_W), np.int64)
        php = np.zeros((128, PACK_W), np.int64)

        def put(off, w, level, tidx, rbit, jbit):
            lvl[:, off:off + w] = level
            tix[:, off:off + w] = np.broadcast_to(tidx, (128, w))
            code = (1 - np.asarray(rbit, np.int64) + np.asarray(jbit, np.int64))
            php[:, off:off + w] = np.broadcast_to(code, (128, w))

        t = np.arange(32)[None, :]
        r = 128 * t + p
        put(OFF["A0"], 32, 0, r & 2047, (r >> 11) & 1, (c >> 2) & 1)
        t16 = np.arange(16)[None, :]
        r16 = 128 * t16 + p
        put(OFF["A1"], 16, 1, (c >> 2) * 1024 + (r16 & 1023),
            (r16 >> 10) & 1, (c >> 1) & 1)
        t8 = np.arange(8)[None, :]
        r8 = 128 * t8 + p
        put(OFF["A2"], 8, 2, (c >> 1) * 512 + (r8 & 511), (r8 >> 9) & 1, c & 1)

        f8 = np.arange(8)[None, :]
        tt = f8 >> 1
        put(OFF["B3"], 8, 3, 256 * c + 128 * (tt & 1) + p, tt >> 1, f8 & 1)
        j7 = f8 & 3
        put(OFF["B4"], 8, 4, (2 * c + (j7 >> 1)) * 128 + p, f8 >> 2, j7 & 1)
        put(OFF["B5"], 8, 5, (4 * c + (f8 >> 1)) * 64 + (p & 63),
            (p >> 6) & 1, f8 & 1)
        for name, i, w, pmask, psh in (
            ("B6", 6, 16, 31, 5), ("B7", 7, 32, 15, 4), ("B8", 8, 64, 7, 3),
            ("B9", 9, 128, 3, 2), ("B10", 10, 256, 1, 1), ("B11", 11, 512, 0, 0),
        ):
            f = np.arange(w)[None, :]
            h = (D >> i) >> 1
            tidx = ((w // 2) * c + (f >> 1)) * h + (p & pmask)
            rbit = (p >> psh) & 1
            put(OFF[name], w, i, tidx, rbit, f & 1)

        lvls.append(lvl)
        tixs.append(tix)
        phps.append(php)
    return lvls, tixs, phps


_LVL, _TIX, _PHP = _build_index_tables()

_TWO_PI = 2.0 * math.pi


def host_input(thetas):
    """Per-core input [128, 1088] f32: F-layout gathered thetas with the
    pi/2 phase folded in, range-wrapped to [-pi, pi] (float64 on host)."""
    outs = []
    for c in range(NCORES):
        arg = thetas[_LVL[c], _TIX[c]].astype(np.float64) + _PHP[c] * HALF_PI
        w = arg - _TWO_PI * np.round(arg / _TWO_PI)
        outs.append(np.ascontiguousarray(w.astype(np.float32)))
    return outs


# ---------------------------------------------------------------------------
# Output tile grouping.  Tile t (output rows 128t..128t+127) uses
# Btt[t & 3] and A column t; with t = 4*A_idx + c the DRAM rows are
# r = 512*A_idx + 128*c + p, so a single-class x A-range group is a
# regular 3D access pattern (the DMA AP balancer rejects 4D).  Groups
# are emitted in expected-ready order, sized small-first so the output
# DMA starts as early as possible and stays saturated; Btt1 is built
# right after the first vector group so the Scalar engine (which owns
# classes 1 and 3-high) starts producing early.  og slots are A-order.
# ---------------------------------------------------------------------------

# (producer, class, a0, a1); t = 4*a + class
GROUPS = (
    ("v", 0, 0, 2),   # t 0,4           ready right after Btt0
    ("v", 0, 2, 4),   # t 8,12
    ("v", 0, 4, 8),   # t 16,20,24,28
    ("v", 2, 0, 8),   # t 2,6,...,30
    ("s", 1, 0, 8),   # t 1,5,...,29
    ("v", 3, 0, 4),   # t 3,7,11,15
    ("s", 3, 4, 8),   # t 19,23,27,31
)
# Btt classes to build (on Vector) right before each group's tiles.
PREBUILD = ((0,), (), (1,), (2, 3), (), (), ())


# ---------------------------------------------------------------------------
# numpy golden model of the on-device pipeline (for testing)
# ---------------------------------------------------------------------------

def golden_core(thetas, c, dtype=np.float32):
    w = host_input(thetas)[c]
    F = np.sin(w.astype(np.float64)).astype(dtype)

    def sl(name, w_):
        o = OFF[name]
        return F[:, o:o + w_]

    # A chain
    a1 = sl("A0", 32) * np.tile(sl("A1", 16), (1, 2))
    A = a1 * np.tile(sl("A2", 8), (1, 4))          # [128, 32], f = t
    # B chain
    G67 = np.repeat(sl("B6", 16), 2, axis=1) * sl("B7", 32)
    G89 = np.repeat(sl("B8", 64), 2, axis=1) * sl("B9", 128)
    G1011 = np.repeat(sl("B10", 256), 2, axis=1) * sl("B11", 512)
    G6789 = np.repeat(G67, 4, axis=1) * G89
    G5_9 = np.repeat(sl("B5", 8), 16, axis=1) * G6789
    H = np.repeat(G5_9, 4, axis=1) * G1011          # [128, 512]
    out = np.empty((D, CPD), dtype)
    B3 = sl("B3", 8)
    B4 = sl("B4", 8)
    Btt = []
    for tt in range(4):
        t34 = np.repeat(B3[:, tt * 2: tt * 2 + 2], 2, axis=1) \
            * B4[:, (tt & 1) * 4: (tt & 1) * 4 + 4]
        Btt.append(np.repeat(t34, 128, axis=1) * H)
    for t in range(32):
        out[128 * t: 128 * (t + 1)] = Btt[t & 3] * A[:, t: t + 1]
    return out


def golden(thetas):
    return np.concatenate([golden_core(thetas, c) for c in range(NCORES)],
                          axis=1)


# ---------------------------------------------------------------------------
# Bass/Tile program
# ---------------------------------------------------------------------------

_NC_CACHE = {}


def make_split_drain_tile_context(sim_mode=False):
    import concourse.tile as tile
    from concourse import mybir

    class SplitDrainTileContext(tile.TileContext):
        """The kernel-tail drain accumulates one sync-wait per outstanding
        semaphore (10+ here); walrus rejects that many wait commands on one
        instruction.  Redistribute them onto single-wait NOPs emitted just
        before the drain (same engine, same program order => identical
        blocking semantics)."""

        def _drain_and_barrier(self, tick_clock, wait_clock):
            from concourse.vector_clock import ScopedClock

            nc = self.nc
            pre_nops = [nc.sync.nop(nofuse=True) for _ in range(30)]
            drain_inst = nc.sync.drain()
            wait_clock.add_sem_waits(
                drain_inst.ins, ScopedClock({None: tick_clock.global_clock})
            )
            di = drain_inst.ins
            si = di.sync_info
            waits = list(si.on_wait) if si is not None and si.on_wait else []
            if len(waits) > 1:
                assert len(waits) <= len(pre_nops), len(waits)
                for w, nop in zip(waits, pre_nops):
                    nop.ins.sync_info = mybir.SyncInfo(on_wait=[w], on_update=[])
                di.sync_info = mybir.SyncInfo(
                    on_wait=[], on_update=list(si.on_update))
            # No all-engine barriers here (the EVSEM butterfly costs ~9us):
            # the drain already guarantees every DMA/engine semaphore
            # reached its final value before SYNC clears them, and the
            # other engines simply halt at the end of their streams.  The
            # clears must run on SYNC (program-ordered after the drain) --
            # the stock clear_and_free_semaphores puts them on gpsimd,
            # which has no ordering against the drain and can clear DMA
            # lane semaphores while output DMAs are still in flight.
            assert self.sems is not None
            popped = nc._tile_sem_poison_stack.pop()
            assert popped is self._sem_poison
            from concourse.bass import compact_to_ranges

            sems = list(self.sems.allocated().values())
            sem_nums = [s.num if hasattr(s, "num") else s for s in sems]
            if not sim_mode:
                # (CoreSim's race detector requires a full barrier before
                # clears; on real HW the sync-engine drain is sufficient
                # ordering.  sim_mode builds skip the clears for value
                # verification.)
                for sem_range in compact_to_ranges(sem_nums):
                    nc.sync.drain(semaphore_range=sem_range)
                    nc.sync.sem_clear(sem_range)
            nc._state.prepend_free_semaphores(sem_nums)
            for poison_set in nc._tile_sem_poison_stack:
                poison_set.update(sem_nums)

    return SplitDrainTileContext


def build_nc(sim_mode=False):
    key = ("nc", sim_mode)
    if key in _NC_CACHE:
        return _NC_CACHE[key]
    from contextlib import ExitStack

    import concourse.bass as bass
    from concourse import mybir

    f32 = mybir.dt.float32
    f16 = mybir.dt.float16
    SplitDrainTileContext = make_split_drain_tile_context(sim_mode)

    nc = bass.Bass()
    pk_d = nc.declare_dram_parameter("pk", [128, PACK_W], f32, isOutput=False)
    out_d = nc.declare_dram_parameter("out", [D, CPD], f32, isOutput=True)

    with SplitDrainTileContext(nc) as tc, ExitStack() as ctx:
        pool = ctx.enter_context(tc.tile_pool(name="main", bufs=1))
        opool = ctx.enter_context(tc.tile_pool(name="out", bufs=1))

        pk = pool.tile([128, PACK_W], f32)
        nc.sync.dma_start(pk[:], pk_d[:, :])

        # F = Sin(pk); the pi/2 phase is folded in and range-reduced on
        # the host, so each chunk is a single activation.  The whole
        # product chain runs in fp16 (2x DVE throughput; worst-case
        # rounding ~4e-3 relative vs the 2e-2 gate); only the final
        # output tiles are written f32.
        F = pool.tile([128, PACK_W], f16)
        for lo, hi in CHUNKS:
            nc.scalar.activation(F[:, lo:hi], pk[:, lo:hi],
                                 mybir.ActivationFunctionType.Sin)

        def sl(name, w):
            o = OFF[name]
            return F[:, o:o + w]

        mult = mybir.AluOpType.mult

        def tt_mul(out_ap, big, small, rep, tiled=False):
            """out = big * expand(small); big [128, W], small [128, W/rep].
            tiled=False: each small elem repeated `rep` consecutive;
            tiled=True: whole small slice repeated `rep` times."""
            w_small = small.shape[1]
            if tiled:
                i1 = small.unsqueeze(1).broadcast_to([128, rep, w_small])
                i0 = big.rearrange("p (a b) -> p a b", a=rep)
                ov = out_ap.rearrange("p (a b) -> p a b", a=rep)
            else:
                i1 = small.unsqueeze(2).broadcast_to([128, w_small, rep])
                i0 = big.rearrange("p (a b) -> p a b", a=w_small)
                ov = out_ap.rearrange("p (a b) -> p a b", a=w_small)
            nc.vector.tensor_tensor(ov, i0, i1, mult)

        # Wide multiply first: G1011 only needs the first Sin chunk, and
        # gates the longest chain, so it runs while chunk 2's Sin is on
        # the Scalar engine.
        G1011 = pool.tile([128, 512], f16)
        tt_mul(G1011[:], sl("B11", 512), sl("B10", 256), 2)

        # Small-multiply chain (needs chunk 2 = B3..B9 + A).
        G67 = pool.tile([128, 32], f16)
        tt_mul(G67[:], sl("B7", 32), sl("B6", 16), 2)
        G89 = pool.tile([128, 128], f16)
        tt_mul(G89[:], sl("B9", 128), sl("B8", 64), 2)
        G6789 = pool.tile([128, 128], f16)
        tt_mul(G6789[:], G89[:], G67[:], 4)
        G5_9 = pool.tile([128, 128], f16)
        tt_mul(G5_9[:], G6789[:], sl("B5", 8), 16)

        # A chain -> A_sb [128, 32] (kept on Vector so every output tile
        # has a single producing engine to wait on)
        a1 = pool.tile([128, 32], f16)
        tt_mul(a1[:], sl("A0", 32), sl("A1", 16), 2, tiled=True)
        # f32: the tensor_scalar / activation per-partition scalar port
        # requires float32.
        A_sb = pool.tile([128, 32], f32)
        tt_mul(A_sb[:], a1[:], sl("A2", 8), 4, tiled=True)

        H = pool.tile([128, 512], f16)
        tt_mul(H[:], G1011[:], G5_9[:], 4)

        # Btt[c] = H * t34[c] broadcast, built on Vector per PREBUILD.
        Btt = [None] * 4

        def build_btt(c):
            t34 = pool.tile([128, 4], f16, tag=f"t34_{c}")
            b3 = sl("B3", 8)[:, c * 2: c * 2 + 2]
            b4 = sl("B4", 8)[:, (c & 1) * 4: (c & 1) * 4 + 4]
            tt_mul(t34[:], b4, b3, 2)
            bt = pool.tile([128, 512], f16, tag=f"Btt_{c}")
            tt_mul(bt[:], H[:], t34[:], 128)
            Btt[c] = bt

        out_v = out_d.rearrange("(A c p) n -> p c A n", c=4, p=128)

        for (eng, cls, a0, a1_), pre in zip(GROUPS, PREBUILD):
            for c in pre:
                build_btt(c)
            ntile = a1_ - a0
            og = opool.tile([128, ntile * CPD], f32, tag=f"og_{cls}_{a0}")
            for q, a in enumerate(range(a0, a1_)):
                t = 4 * a + cls
                ot = og[:, q * CPD:(q + 1) * CPD]
                if eng == "v":
                    nc.vector.tensor_scalar_mul(ot, Btt[cls][:],
                                                A_sb[:, t: t + 1])
                else:
                    nc.scalar.mul(ot, Btt[cls][:], A_sb[:, t: t + 1])
            dram = out_v[:, cls, a0:a1_, :]
            nc.sync.dma_start(
                dram, og[:].rearrange("p (a n) -> p a n", a=ntile))

    _NC_CACHE[key] = nc
    return nc


def kernel(thetas):
    thetas = np.asarray(thetas, np.float32)
    assert thetas.shape == (M, D // 2)
    from concourse.bass_utils import run_bass_kernel_spmd

    nc = build_nc()
    packs = host_input(thetas)
    in_maps = [{"pk": packs[c]} for c in range(NCORES)]
    res = run_bass_kernel_spmd(nc, in_maps, core_ids=list(range(NCORES)))
    return np.concatenate([res.results[c]["out"] for c in range(NCORES)],
                          axis=1)


if __name__ == "__main__":
    # quick self-check of golden vs closed form
    rng = np.random.RandomState(0)
    th = rng.randn(M, D // 2).astype(np.float32)
    r = np.arange(D)[:, None]
    j = np.arange(D)[None, :]
    R = np.ones((D, D))
    for i in range(M):
        k = D >> i
        h = k >> 1
        rbit = (r // h) & 1
        jbit = (j // h) & 1
        tidx = (j // k) * h + (r % h)
        thl = th[i][tidx].astype(np.float64)
        Fm = np.where(rbit == jbit, np.cos(thl),
                      np.where(rbit == 1, np.sin(thl), -np.sin(thl)))
        R *= Fm
    G = golden(th).astype(np.float64)
    err = np.abs(R - G).max()
    print("golden vs closed-form max abs err:", err)
    assert err < 1e-5, err
    print("OK")
